# revision 10
# baseline (speedup 1.0000x reference)
"""Fused masked-softmax attention (DotProductAttention) for 8 TRN2 NeuronCores.

Problem: B=16 batches of Q[2048,64] @ K[2048,64]^T -> mask cols >= valid_len
to -1e6 -> softmax -> @ V[2048,64].  Data-parallel: 2 batches per core.

Per-core kernel design (all on-chip, scores never touch HBM):
  * Layout: S^T[k, q] so softmax's k-reduction becomes a matmul and the
    attn @ V contraction needs no transpose of the big matrix.
  * mm1:  S^T chunk [128k, 512q] = kTa[:, ktile].T @ qTa[:, qtile] with
    AUGMENTED bf16 operands: kTa = [K^T; mask_row] (65 rows), qTa =
    [Q^T; ones] (65 rows).  The 65th contraction row adds -8e6 to every
    masked column, so masking costs zero extra instructions.  bf16 streams
    1 row/cycle on the PE (fp32 is 4x, fp32r 2x).
  * exp:  ACT engine (the bottleneck, ~1 elem/lane/cycle @1.2GHz),
    exp(0.125 * x) straight out of PSUM in merged N=1536 activations
    (per-instruction overhead ~350 cycles), bf16 out.
    exp(0.125*(s - 8e6)) == 0 for masked rows.
  * mm2:  O^T_aug [65, 512q] = sum_k Vaug[ktile].T @ expS^T[ktile] where
    Vaug = [V | ones] (bf16) -> row 64 accumulates the softmax denominator
    in fp32 PSUM.  Interleaved group-by-group with mm1/exp so the PE
    pipeline has no serial tail.
  * finish: copy PSUM->SBUF (f32), PE-transpose 128-col chunks back to
    [q, d] layout, reciprocal of denominator column, per-partition scale,
    one merged DMA per 512-row q-tile.
"""

import functools

import numpy as np
import ml_dtypes

import concourse.bacc as bacc
import concourse.tile as tile
from concourse import mybir
from concourse import bass_utils
from concourse.masks import make_identity

B, LQ, LKV, D = 16, 2048, 2048, 64
N_CORES = 8
BPC = B // N_CORES  # batches per core
KT = 128            # k-tile (partition dim of S^T)
QT = 512            # q-tile (free dim / PSUM bank)
NKT = LKV // KT     # 16
NQT = LQ // QT      # 4
GROUP = 3           # k-tiles per PSUM tile / merged activation
MASK_RAW = -8.0e6   # * 0.125 scale == -1e6 (reference MASK_VALUE)
F32 = mybir.dt.float32
BF16 = mybir.dt.bfloat16


@functools.lru_cache(maxsize=1)
def _build_module():
    nc = bacc.Bacc(None)
    qta_d = nc.dram_tensor("qta", [BPC, D + 1, LQ], BF16, kind="ExternalInput")
    kta_d = nc.dram_tensor("kta", [BPC, D + 1, LKV], BF16, kind="ExternalInput")
    vau_d = nc.dram_tensor("vaug", [128, BPC * NKT * (D + 1)], BF16, kind="ExternalInput")
    out_d = nc.dram_tensor("o", [BPC, LQ, D], F32, kind="ExternalOutput")

    # First group is 1 k-tile wide so the first exp can start as soon as one
    # 16KB DMA chunk + one matmul complete; the rest are GROUP wide.
    groups = [(0, 1)]
    g = 1
    while g < NKT:
        groups.append((g, min(GROUP, NKT - g)))
        g += GROUP

    with tile.TileContext(nc) as tc:
        with (
            tc.tile_pool(name="weights", bufs=1) as wpool,
            tc.tile_pool(name="exps", bufs=2) as epool,
            tc.tile_pool(name="ot", bufs=2) as otpool,
            tc.tile_pool(name="recip", bufs=2) as rpool,
            tc.tile_pool(name="outs", bufs=2) as opool,
            tc.tile_pool(name="ps_s", bufs=2, space="PSUM") as ps_s,
            tc.tile_pool(name="ps_o", bufs=1, space="PSUM") as ps_o,
            tc.tile_pool(name="ps_t", bufs=1, space="PSUM") as ps_t,
        ):
            ident = wpool.tile([128, 128], F32, tag="ident")
            make_identity(nc, ident)

            # Input loads, chunked so the first matmul group can start as
            # early as possible (issue order == consumption order).
            kta_s = [
                wpool.tile([D + 1, LKV], BF16, tag=f"kta{i}", name=f"kta{i}")
                for i in range(BPC)
            ]
            qta_s = [
                wpool.tile([D + 1, LQ], BF16, tag=f"qta{i}", name=f"qta{i}")
                for i in range(BPC)
            ]
            vaug_s = wpool.tile([128, BPC * NKT * (D + 1)], BF16, tag="vaug")
            nc.sync.dma_start(out=kta_s[0][:, :KT], in_=kta_d[0, :, :KT])
            nc.sync.dma_start(out=qta_s[0][:, :QT], in_=qta_d[0, :, :QT])
            nc.sync.dma_start(out=kta_s[0][:, KT:], in_=kta_d[0, :, KT:])
            nc.sync.dma_start(out=vaug_s[:, : NKT * (D + 1)], in_=vau_d[:, : NKT * (D + 1)])
            nc.sync.dma_start(out=qta_s[0][:, QT:], in_=qta_d[0, :, QT:])
            nc.sync.dma_start(out=kta_s[1], in_=kta_d[1])
            nc.sync.dma_start(out=qta_s[1], in_=qta_d[1])
            nc.sync.dma_start(out=vaug_s[:, NKT * (D + 1) :], in_=vau_d[:, NKT * (D + 1) :])

            def finish(i, qi, po):
                """Normalize po [65, 512] and store as out[i, qi*512:+512, :]."""
                ot = otpool.tile([D + 1, QT], F32, tag="ot", name="ot")
                nc.vector.tensor_copy(ot, po)
                pt = ps_t.tile([128, QT // 128, D + 1], F32, tag="pt", name="pt")
                for j in range(QT // 128):
                    nc.tensor.transpose(
                        pt[:, j, :],
                        ot[:, j * 128 : (j + 1) * 128],
                        ident[: D + 1, : D + 1],
                    )
                rc = rpool.tile([128, QT // 128], F32, tag="rc", name="rc")
                nc.vector.reciprocal(rc, pt[:, :, D])
                ob = opool.tile([128, QT // 128, D], F32, tag="ob", name="ob")
                for j in range(QT // 128):
                    nc.vector.tensor_scalar_mul(
                        ob[:, j, :], pt[:, j, :D], rc[:, j : j + 1]
                    )
                out_ap = out_d[i, qi * QT : (qi + 1) * QT, :].rearrange(
                    "(j p) d -> p j d", p=128
                )
                nc.sync.dma_start(out=out_ap, in_=ob)

            # Flat software-pipelined stream over (batch, qtile, group): the
            # mm2 stage lags mm1/exp by one group so the PE always has the
            # NEXT qtile's mm1 work queued before the previous qtile's last
            # mm2 — ACT never waits at qtile boundaries and there is no
            # serial tail per qtile.
            seq = [(i, qi, g, w) for i in range(BPC) for qi in range(NQT) for g, w in groups]
            exps_t = {}
            po_t = {}
            pending = None  # (i, qi, g, w) whose mm2 hasn't been emitted

            def mm2(i, qi, g, w):
                po = po_t[(i, qi)]
                exps = exps_t[(i, qi)]
                for j in range(w):
                    n = g + j
                    base = (i * NKT + n) * (D + 1)
                    nc.tensor.matmul(
                        po,
                        lhsT=vaug_s[:, base : base + D + 1],
                        rhs=exps[:, n * QT : (n + 1) * QT],
                        start=(n == 0),
                        stop=(n == NKT - 1),
                        skip_group_check=True,
                    )
                if g + w == NKT:
                    finish(i, qi, po)
                    del po_t[(i, qi)], exps_t[(i, qi)]

            for i, qi, g, w in seq:
                if g == 0:
                    exps_t[(i, qi)] = epool.tile(
                        [128, NKT * QT], BF16, tag="exps", name="exps"
                    )
                    po_t[(i, qi)] = ps_o.tile([D + 1, QT], F32, tag="po", name="po")
                rhs = qta_s[i][:, qi * QT : (qi + 1) * QT]
                st = ps_s.tile([128, GROUP * QT], F32, tag="st", name="st")
                for j in range(w):
                    n = g + j
                    nc.tensor.matmul(
                        st[:, j * QT : (j + 1) * QT],
                        lhsT=kta_s[i][:, n * KT : (n + 1) * KT],
                        rhs=rhs,
                        start=True,
                        stop=True,
                    )
                nc.scalar.activation(
                    out=exps_t[(i, qi)][:, g * QT : (g + w) * QT],
                    in_=st[:, : w * QT],
                    func=mybir.ActivationFunctionType.Exp,
                    scale=0.125,
                )
                if pending is not None:
                    mm2(*pending)
                pending = (i, qi, g, w)
            mm2(*pending)

    nc.compile()
    return nc


def _shard_inputs(queries, keys, values, valid_lens):
    """Host-side layout: augmented transposed Q/K, tiled V|ones, bf16."""
    Q = np.asarray(queries, dtype=np.float32)
    K = np.asarray(keys, dtype=np.float32)
    V = np.asarray(values, dtype=np.float32)
    VL = np.asarray(valid_lens).astype(np.int64)

    cols = np.arange(LKV, dtype=np.int64)
    in_maps = []
    for c in range(N_CORES):
        bs = slice(BPC * c, BPC * (c + 1))
        qta = np.concatenate(
            [Q[bs].transpose(0, 2, 1), np.ones((BPC, 1, LQ), np.float32)], axis=1
        )
        mask = np.where(cols[None, :] >= VL[bs][:, None], MASK_RAW, 0.0).astype(
            np.float32
        )
        kta = np.concatenate([K[bs].transpose(0, 2, 1), mask[:, None, :]], axis=1)
        # V|ones -> [BPC, NKT, 128, 65] -> partition-major [128, BPC*NKT*65]
        va = np.concatenate([V[bs], np.ones((BPC, LKV, 1), np.float32)], axis=-1)
        va = va.reshape(BPC, NKT, KT, D + 1).transpose(2, 0, 1, 3).reshape(128, -1)
        in_maps.append(
            {
                "qta": np.ascontiguousarray(qta).astype(ml_dtypes.bfloat16),
                "kta": np.ascontiguousarray(kta).astype(ml_dtypes.bfloat16),
                "vaug": np.ascontiguousarray(va).astype(ml_dtypes.bfloat16),
            }
        )
    return in_maps


def kernel(queries, keys, values, valid_lens):
    nc = _build_module()
    in_maps = _shard_inputs(queries, keys, values, valid_lens)
    res = bass_utils.run_bass_kernel_spmd(nc, in_maps, core_ids=list(range(N_CORES)))
    out = np.concatenate([r["o"] for r in res.results], axis=0)
    return out.reshape(B, LQ, D).astype(np.float32)


# revision 13
# speedup vs baseline: 1.2633x; 1.2633x over previous
"""Fused masked-softmax attention (DotProductAttention) for 8 TRN2 NeuronCores.

Problem: B=16 batches of Q[2048,64] @ K[2048,64]^T -> mask cols >= valid_len
to -1e6 -> softmax -> @ V[2048,64].  Data-parallel: 2 batches per core.

Per-core kernel design (all on-chip, scores never touch HBM):
  * Layout: S^T[k, q] so softmax's k-reduction becomes a matmul and the
    attn @ V contraction needs no transpose of the big matrix.
  * mm1:  S^T chunk [128k, 512q] = kTa[:, ktile].T @ qTa[:, qtile] with
    AUGMENTED bf16 operands: kTa = [K^T; mask_row] (65 rows), qTa =
    [Q^T; ones] (65 rows).  The 65th contraction row adds -8e6 to every
    masked column, so masking costs zero extra instructions.  bf16 streams
    1 row/cycle on the PE (fp32 is 4x, fp32r 2x).
  * exp:  ACT engine (the bottleneck, ~1 elem/lane/cycle @1.2GHz),
    exp(0.125 * x) straight out of PSUM in merged N=1536 activations
    (per-instruction overhead ~350 cycles), bf16 out.
    exp(0.125*(s - 8e6)) == 0 for masked rows.
  * mm2:  O^T_aug [65, 512q] = sum_k Vaug[ktile].T @ expS^T[ktile] where
    Vaug = [V | ones] (bf16) -> row 64 accumulates the softmax denominator
    in fp32 PSUM.  Interleaved group-by-group with mm1/exp so the PE
    pipeline has no serial tail.
  * finish: copy PSUM->SBUF (f32), PE-transpose 128-col chunks back to
    [q, d] layout, reciprocal of denominator column, per-partition scale,
    one merged DMA per 512-row q-tile.
"""

import functools

import numpy as np
import ml_dtypes

import concourse.bacc as bacc
import concourse.tile as tile
from concourse import mybir
from concourse import bass_utils
from concourse.masks import make_identity

B, LQ, LKV, D = 16, 2048, 2048, 64
N_CORES = 8
BPC = B // N_CORES  # batches per core
KT = 128            # k-tile (partition dim of S^T)
QT = 512            # q-tile (free dim / PSUM bank)
NKT = LKV // KT     # 16
NQT = LQ // QT      # 4
GROUP = 3           # k-tiles per PSUM tile / merged activation
MASK_RAW = -8.0e6   # * 0.125 scale == -1e6 (reference MASK_VALUE)
F32 = mybir.dt.float32
BF16 = mybir.dt.bfloat16


@functools.lru_cache(maxsize=1)
def _build_module():
    nc = bacc.Bacc(None)
    qta_d = nc.dram_tensor("qta", [BPC, D + 1, LQ], BF16, kind="ExternalInput")
    kta_d = nc.dram_tensor("kta", [BPC, D + 1, LKV], BF16, kind="ExternalInput")
    vau_d = nc.dram_tensor("vaug", [128, BPC * NKT * (D + 1)], BF16, kind="ExternalInput")
    out_d = nc.dram_tensor("o", [BPC, LQ, D], F32, kind="ExternalOutput")

    # k-tile groups per PSUM tile / merged activation: [3,3,3,3,2,2] balances
    # large merged activations against the 3-bank PSUM tile limit.
    widths = [3, 3, 3, 3, 2, 2]
    groups, g = [], 0
    for w in widths:
        groups.append((g, w))
        g += w
    assert g == NKT

    with tile.TileContext(nc) as tc:
        with (
            tc.tile_pool(name="weights", bufs=1) as wpool,
            tc.tile_pool(name="exps", bufs=2) as epool,
            tc.tile_pool(name="ot", bufs=2) as otpool,
            tc.tile_pool(name="recip", bufs=2) as rpool,
            tc.tile_pool(name="outs", bufs=2) as opool,
            tc.tile_pool(name="ps_s", bufs=2, space="PSUM") as ps_s,
            tc.tile_pool(name="ps_o", bufs=1, space="PSUM") as ps_o,
            tc.tile_pool(name="ps_t", bufs=1, space="PSUM") as ps_t,
        ):
            ident = wpool.tile([128, 128], F32, tag="ident")
            make_identity(nc, ident)

            # Input loads, chunked so the first matmul group can start as
            # early as possible (issue order == consumption order).
            kta_s = [
                wpool.tile([D + 1, LKV], BF16, tag=f"kta{i}", name=f"kta{i}")
                for i in range(BPC)
            ]
            qta_s = [
                wpool.tile([D + 1, LQ], BF16, tag=f"qta{i}", name=f"qta{i}")
                for i in range(BPC)
            ]
            vaug_s = wpool.tile([128, BPC * NKT * (D + 1)], BF16, tag="vaug")
            c0 = widths[0] * KT
            nc.sync.dma_start(out=kta_s[0][:, :c0], in_=kta_d[0, :, :c0])
            nc.sync.dma_start(out=qta_s[0][:, :QT], in_=qta_d[0, :, :QT])
            nc.sync.dma_start(out=kta_s[0][:, c0:], in_=kta_d[0, :, c0:])
            nc.sync.dma_start(out=vaug_s[:, : NKT * (D + 1)], in_=vau_d[:, : NKT * (D + 1)])
            nc.sync.dma_start(out=qta_s[0][:, QT:], in_=qta_d[0, :, QT:])
            nc.sync.dma_start(out=kta_s[1], in_=kta_d[1])
            nc.sync.dma_start(out=qta_s[1], in_=qta_d[1])
            nc.sync.dma_start(out=vaug_s[:, NKT * (D + 1) :], in_=vau_d[:, NKT * (D + 1) :])

            def finish(i, qi, po):
                """Normalize po [65, 512] and store as out[i, qi*512:+512, :]."""
                ot = otpool.tile([D + 1, QT], F32, tag="ot", name="ot")
                nc.vector.tensor_copy(ot, po)
                pt = ps_t.tile([128, QT // 128, D + 1], F32, tag="pt", name="pt")
                for j in range(QT // 128):
                    nc.tensor.transpose(
                        pt[:, j, :],
                        ot[:, j * 128 : (j + 1) * 128],
                        ident[: D + 1, : D + 1],
                    )
                rc = rpool.tile([128, QT // 128], F32, tag="rc", name="rc")
                nc.vector.reciprocal(rc, pt[:, :, D])
                ob = opool.tile([128, QT // 128, D], F32, tag="ob", name="ob")
                for j in range(QT // 128):
                    nc.vector.tensor_scalar_mul(
                        ob[:, j, :], pt[:, j, :D], rc[:, j : j + 1]
                    )
                out_ap = out_d[i, qi * QT : (qi + 1) * QT, :].rearrange(
                    "(j p) d -> p j d", p=128
                )
                nc.sync.dma_start(out=out_ap, in_=ob)

            for i in range(BPC):
                for qi in range(NQT):
                    rhs = qta_s[i][:, qi * QT : (qi + 1) * QT]
                    exps = epool.tile([128, NKT * QT], BF16, tag="exps", name="exps")
                    po = ps_o.tile([D + 1, QT], F32, tag="po", name="po")
                    for g, w in groups:
                        st = ps_s.tile([128, GROUP * QT], F32, tag="st", name="st")
                        for j in range(w):
                            n = g + j
                            nc.tensor.matmul(
                                st[:, j * QT : (j + 1) * QT],
                                lhsT=kta_s[i][:, n * KT : (n + 1) * KT],
                                rhs=rhs,
                                start=True,
                                stop=True,
                            )
                        nc.scalar.activation(
                            out=exps[:, g * QT : (g + w) * QT],
                            in_=st[:, : w * QT],
                            func=mybir.ActivationFunctionType.Exp,
                            scale=0.125,
                        )
                        for j in range(w):
                            n = g + j
                            base = (i * NKT + n) * (D + 1)
                            nc.tensor.matmul(
                                po,
                                lhsT=vaug_s[:, base : base + D + 1],
                                rhs=exps[:, n * QT : (n + 1) * QT],
                                start=(n == 0),
                                stop=(n == NKT - 1),
                                skip_group_check=True,
                            )
                    finish(i, qi, po)

    nc.compile()
    return nc


def _shard_inputs(queries, keys, values, valid_lens):
    """Host-side layout: augmented transposed Q/K, tiled V|ones, bf16."""
    Q = np.asarray(queries, dtype=np.float32)
    K = np.asarray(keys, dtype=np.float32)
    V = np.asarray(values, dtype=np.float32)
    VL = np.asarray(valid_lens).astype(np.int64)

    cols = np.arange(LKV, dtype=np.int64)
    in_maps = []
    for c in range(N_CORES):
        bs = slice(BPC * c, BPC * (c + 1))
        qta = np.concatenate(
            [Q[bs].transpose(0, 2, 1), np.ones((BPC, 1, LQ), np.float32)], axis=1
        )
        mask = np.where(cols[None, :] >= VL[bs][:, None], MASK_RAW, 0.0).astype(
            np.float32
        )
        kta = np.concatenate([K[bs].transpose(0, 2, 1), mask[:, None, :]], axis=1)
        # V|ones -> [BPC, NKT, 128, 65] -> partition-major [128, BPC*NKT*65]
        va = np.concatenate([V[bs], np.ones((BPC, LKV, 1), np.float32)], axis=-1)
        va = va.reshape(BPC, NKT, KT, D + 1).transpose(2, 0, 1, 3).reshape(128, -1)
        in_maps.append(
            {
                "qta": np.ascontiguousarray(qta).astype(ml_dtypes.bfloat16),
                "kta": np.ascontiguousarray(kta).astype(ml_dtypes.bfloat16),
                "vaug": np.ascontiguousarray(va).astype(ml_dtypes.bfloat16),
            }
        )
    return in_maps


def kernel(queries, keys, values, valid_lens):
    nc = _build_module()
    in_maps = _shard_inputs(queries, keys, values, valid_lens)
    res = bass_utils.run_bass_kernel_spmd(nc, in_maps, core_ids=list(range(N_CORES)))
    out = np.concatenate([r["o"] for r in res.results], axis=0)
    return out.reshape(B, LQ, D).astype(np.float32)
